# revision 6
# baseline (speedup 1.0000x reference)
"""Multi-head attention (B=2, S=2048, D=1024, H=16) on 8 trn2 NeuronCores.

Sharding: (batch, head-group) -> core.  Core c handles batch b=c//4 and the
4 heads [4*(c%4), 4*(c%4)+4).  Per core:
  - QKV projections for its head slice (bias folded in via an augmented
    ones-row matmul; the 1/sqrt(depth) logit scale folded into wq).
  - Attention with logits computed TRANSPOSED (logitsT[k,q]) so that
    PT = exp(logitsT) feeds the ctx matmul directly as the stationary
    operand with zero on-chip transposes.
  - Softmax denominators come for free from a ones-column appended to V
    (row 64 of the ctx accumulator = sum_k exp(logits[k,q])).
  - attn probabilities are written as attnT[h,k,q] (perfectly contiguous
    DMA); the host transposes at gather time.
  - Output projection produces a partial out (this core's heads); host sums
    the 4 partials per batch.
"""

import os
import sys
from contextlib import ExitStack

import numpy as np

for _p in ("/opt/trn_rl_repo", "/root/.axon_site/_ro/trn_rl_repo"):
    if os.path.isdir(_p) and _p not in sys.path:
        sys.path.insert(0, _p)

import concourse.bass as bass  # noqa: E402
import concourse.mybir as mybir  # noqa: E402
import concourse.tile as tile  # noqa: E402
from concourse import bacc  # noqa: E402
from concourse.bass_utils import run_bass_kernel_spmd  # noqa: E402

F32 = mybir.dt.float32
EXP = mybir.ActivationFunctionType.Exp
COPY = mybir.ActivationFunctionType.Copy

S = 2048          # sequence length
DM = 1024         # d_model
DH = 64           # depth per head
NH = 16           # total heads
HPC = 4           # heads per core
KT = DM // 128    # 8 k-tiles over d_model
NS = S // 128     # 16 s-tiles (128-row blocks)
QH = S // 2       # q processed in two halves of 1024
N_CORES = 8

_compiled_nc = None
last_results = None  # BassKernelResults of the most recent run (for test.py)


def _build(reps=1):
    nc = bacc.Bacc(None)

    xqT = nc.dram_tensor("xqT", [DM, S], F32, kind="ExternalInput")
    xkT = nc.dram_tensor("xkT", [DM, S], F32, kind="ExternalInput")
    xvT = nc.dram_tensor("xvT", [DM, S], F32, kind="ExternalInput")
    wq_s = nc.dram_tensor("wq_s", [DM + 1, HPC * DH], F32, kind="ExternalInput")
    wk_s = nc.dram_tensor("wk_s", [DM + 1, HPC * DH], F32, kind="ExternalInput")
    wv_s = nc.dram_tensor("wv_s", [DM + 1, HPC * DH], F32, kind="ExternalInput")
    wo_s = nc.dram_tensor("wo_s", [HPC * DH + 1, DM], F32, kind="ExternalInput")
    attnT = nc.dram_tensor("attnT", [HPC, S, S], F32, kind="ExternalOutput")
    outp = nc.dram_tensor("outp", [S, DM], F32, kind="ExternalOutput")
    qk_spill = nc.dram_tensor("qk_spill", [2, HPC, DH, S], F32)

    with tile.TileContext(nc) as tc:
      for _rep in range(reps):
       with ExitStack() as ctx:
        # ---------- persistent pools ----------
        vones_p = ctx.enter_context(tc.tile_pool(name="vones", bufs=NS))
        ctxT_p = ctx.enter_context(tc.tile_pool(name="ctxT", bufs=2))
        ones_p = ctx.enter_context(tc.tile_pool(name="ones", bufs=1))

        # V (natural layout) with a ones column per head: [128, 4*(64+1)]
        vones = [vones_p.tile([128, HPC * (DH + 1)], F32, tag="vones", name="vones")
                 for _ in range(NS)]
        # ctxT (unnormalized attn-weighted values, transposed): 2 x [128, S]
        ctxT = [ctxT_p.tile([128, S], F32, tag="ctxT", name="ctxT") for _ in range(2)]
        ones1 = ones_p.tile([1, 128], F32, tag="ones", name="ones1")
        nc.vector.memset(ones1, 1.0)

        # ---------- phase 1: projections ----------
        with tc.tile_pool(name="xT", bufs=9) as xT_p, \
             tc.tile_pool(name="wt", bufs=10) as wt_p, \
             tc.tile_pool(name="stg", bufs=4) as stg_p, \
             tc.tile_pool(name="orow", bufs=1) as orow_p, \
             tc.tile_pool(name="pj", bufs=8, space="PSUM") as pj_ps:

            ones_row = orow_p.tile([1, S], F32, tag="orow", name="ones_row")
            nc.vector.memset(ones_row, 1.0)

            # --- Q and K projections -> qhT/khT [64, S] per head, spilled ---
            for which, xsrc, wsrc in ((0, xqT, wq_s), (1, xkT, wk_s)):
                wts, xts = [], []
                for kk in range(KT + 1):
                    rows = 128 if kk < KT else 1
                    wt = wt_p.tile([rows, HPC * DH], F32, tag="wt", name="wt")
                    nc.sync.dma_start(out=wt, in_=wsrc[kk * 128:kk * 128 + rows, :])
                    wts.append(wt)
                for kk in range(KT):
                    xt = xT_p.tile([128, S], F32, tag="xT", name="xt")
                    nc.sync.dma_start(out=xt, in_=xsrc[kk * 128:(kk + 1) * 128, :])
                    xts.append(xt)
                xts.append(ones_row)

                pss = [[pj_ps.tile([128, 512], F32, tag="pj", name="pjq") for _ in range(4)]
                       for _ in range(2)]
                for kk in range(KT + 1):
                    for m in range(2):
                        for n in range(4):
                            nc.tensor.matmul(
                                pss[m][n],
                                lhsT=wts[kk][:, m * 128:(m + 1) * 128],
                                rhs=xts[kk][:, n * 512:(n + 1) * 512],
                                start=(kk == 0), stop=(kk == KT))
                stgs = [stg_p.tile([DH, S], F32, tag="stg", name="stg") for _ in range(HPC)]
                for m in range(2):
                    for n in range(4):
                        ns_ = slice(n * 512, (n + 1) * 512)
                        nc.vector.tensor_copy(stgs[2 * m][:, ns_], pss[m][n][0:DH, :])
                        nc.vector.tensor_copy(stgs[2 * m + 1][:, ns_], pss[m][n][DH:128, :])
                for h in range(HPC):
                    nc.sync.dma_start(out=qk_spill[which, h], in_=stgs[h])

            # --- V projection -> vones tiles (natural [S, 4*64] + ones) ---
            wvs, xvs = [], []
            for kk in range(KT + 1):
                rows = 128 if kk < KT else 1
                wt = wt_p.tile([rows, HPC * DH], F32, tag="wt", name="wt")
                nc.sync.dma_start(out=wt, in_=wv_s[kk * 128:kk * 128 + rows, :])
                wvs.append(wt)
            for kk in range(KT):
                xt = xT_p.tile([128, S], F32, tag="xT", name="xt")
                nc.sync.dma_start(out=xt, in_=xvT[kk * 128:(kk + 1) * 128, :])
                xvs.append(xt)
            xvs.append(ones_row)

            for s in range(NS):
                nc.vector.memset(vones[s], 1.0)
                ps = pj_ps.tile([128, HPC * DH], F32, tag="pj", name="pjv")
                for kk in range(KT + 1):
                    nc.tensor.matmul(
                        ps,
                        lhsT=xvs[kk][:, s * 128:(s + 1) * 128],
                        rhs=wvs[kk],
                        start=(kk == 0), stop=(kk == KT))
                dst = vones[s].rearrange("p (h c) -> p h c", c=DH + 1)[:, :, 0:DH]
                src = ps.rearrange("p (h c) -> p h c", c=DH)
                nc.vector.tensor_copy(dst, src)

        # ---------- phase 2: attention ----------
        with tc.tile_pool(name="qk", bufs=4) as qk_p, \
             tc.tile_pool(name="PT", bufs=16) as PT_p, \
             tc.tile_pool(name="rows", bufs=2) as rows_p, \
             tc.tile_pool(name="bc", bufs=2) as bc_p, \
             tc.tile_pool(name="lg", bufs=2, space="PSUM") as lg_ps, \
             tc.tile_pool(name="cx", bufs=2, space="PSUM") as cx_ps:

            for h in range(HPC):
                qh = qk_p.tile([DH, S], F32, tag="qk", name="qk")
                kh = qk_p.tile([DH, S], F32, tag="qk", name="qk")
                nc.sync.dma_start(out=qh, in_=qk_spill[0, h])
                nc.sync.dma_start(out=kh, in_=qk_spill[1, h])
                for q2 in range(2):
                    q0 = q2 * QH
                    cx = cx_ps.tile([DH + 1, QH], F32, tag="cx", name="cx")
                    pts = []
                    for kb in range(NS):
                        lg = lg_ps.tile([128, QH], F32, tag="lg", name="lg")
                        for n in range(2):
                            nc.tensor.matmul(
                                lg[:, n * 512:(n + 1) * 512],
                                lhsT=kh[:, kb * 128:(kb + 1) * 128],
                                rhs=qh[:, q0 + n * 512:q0 + (n + 1) * 512],
                                start=True, stop=True)
                        pt = PT_p.tile([128, QH], F32, tag="PT", name="pt")
                        nc.scalar.activation(pt, lg, EXP)
                        for n in range(2):
                            nc.tensor.matmul(
                                cx[:, n * 512:(n + 1) * 512],
                                lhsT=vones[kb][:, h * (DH + 1):(h + 1) * (DH + 1)],
                                rhs=pt[:, n * 512:(n + 1) * 512],
                                start=(kb == 0), stop=(kb == NS - 1))
                        pts.append(pt)

                    # denominators -> reciprocal row -> broadcast tile
                    rcp = rows_p.tile([1, QH], F32, tag="rows", name="rcp")
                    nc.vector.reciprocal(rcp, cx[DH:DH + 1, :])
                    bps = lg_ps.tile([128, QH], F32, tag="lg", name="lg")
                    for n in range(2):
                        nc.tensor.matmul(
                            bps[:, n * 512:(n + 1) * 512],
                            lhsT=ones1,
                            rhs=rcp[:, n * 512:(n + 1) * 512],
                            start=True, stop=True)
                    bc = bc_p.tile([128, QH], F32, tag="bc", name="bc")
                    nc.vector.tensor_copy(bc, bps)

                    # ctxT rows: fused normalize + copyback into resident tile
                    t, r0 = h // 2, (h % 2) * DH
                    nc.vector.tensor_mul(
                        ctxT[t][r0:r0 + DH, q0:q0 + QH], cx[0:DH, :], bc[0:DH, :])

                    # normalize PT in place and stream out
                    for kb in range(NS):
                        nc.vector.tensor_mul(pts[kb], pts[kb], bc)
                        nc.sync.dma_start(
                            out=attnT[h, kb * 128:(kb + 1) * 128, q0:q0 + QH],
                            in_=pts[kb])

        # ---------- phase 3: output projection (partial over this core's heads) ----------
        with tc.tile_pool(name="wo", bufs=3) as wo_p, \
             tc.tile_pool(name="ost", bufs=3) as ost_p, \
             tc.tile_pool(name="ops", bufs=4, space="PSUM") as o_ps:
            wo0 = wo_p.tile([128, DM], F32, tag="wo", name="wo")
            wo1 = wo_p.tile([128, DM], F32, tag="wo", name="wo")
            wob = wo_p.tile([1, DM], F32, tag="wo", name="wob")
            nc.sync.dma_start(out=wo0, in_=wo_s[0:128, :])
            nc.sync.dma_start(out=wo1, in_=wo_s[128:256, :])
            nc.sync.dma_start(out=wob, in_=wo_s[256:257, :])

            for m in range(NS):
                ms = slice(m * 128, (m + 1) * 128)
                ost = ost_p.tile([128, DM], F32, tag="ost", name="ost")
                pss = [o_ps.tile([128, 512], F32, tag="ops", name="ops") for _ in range(2)]
                for ti, (lhs_full, rhs_t) in enumerate(
                        ((ctxT[0], wo0), (ctxT[1], wo1), (ones1, wob))):
                    lhs = lhs_full if ti == 2 else lhs_full[:, ms]
                    for n in range(2):
                        nc.tensor.matmul(
                            pss[n],
                            lhsT=lhs,
                            rhs=rhs_t[:, n * 512:(n + 1) * 512],
                            start=(ti == 0), stop=(ti == 2))
                for n in range(2):
                    nc.scalar.activation(ost[:, n * 512:(n + 1) * 512], pss[n], COPY)
                nc.sync.dma_start(out=outp[ms, :], in_=ost)

    nc.compile()
    return nc


def _get_nc():
    global _compiled_nc
    if _compiled_nc is None:
        _compiled_nc = _build()
    return _compiled_nc


def prepare_in_maps(q, k, v, wq, bq, wk, bk, wv, bv, wo, bo):
    q = np.asarray(q, np.float32)
    k = np.asarray(k, np.float32)
    v = np.asarray(v, np.float32)
    wq = np.asarray(wq, np.float32)
    bq = np.asarray(bq, np.float32)
    wk = np.asarray(wk, np.float32)
    bk = np.asarray(bk, np.float32)
    wv = np.asarray(wv, np.float32)
    bv = np.asarray(bv, np.float32)
    wo = np.asarray(wo, np.float32)
    bo = np.asarray(bo, np.float32)
    B = q.shape[0]
    scale = np.float32(1.0 / np.sqrt(DH))

    xT = {}
    for b in range(B):
        xT[("q", b)] = np.ascontiguousarray(q[b].T)
        xT[("k", b)] = np.ascontiguousarray(k[b].T)
        xT[("v", b)] = np.ascontiguousarray(v[b].T)

    in_maps = []
    for c in range(N_CORES):
        b, g = c // 4, c % 4
        col = g * HPC * DH
        cs = slice(col, col + HPC * DH)
        wq_c = np.concatenate([wq[:, cs], bq[None, cs]], 0) * scale
        wk_c = np.concatenate([wk[:, cs], bk[None, cs]], 0)
        wv_c = np.concatenate([wv[:, cs], bv[None, cs]], 0)
        bo_row = bo[None, :] if g == 0 else np.zeros((1, DM), np.float32)
        wo_c = np.concatenate([wo[cs, :], bo_row], 0)
        in_maps.append({
            "xqT": xT[("q", b)],
            "xkT": xT[("k", b)],
            "xvT": xT[("v", b)],
            "wq_s": np.ascontiguousarray(wq_c),
            "wk_s": np.ascontiguousarray(wk_c),
            "wv_s": np.ascontiguousarray(wv_c),
            "wo_s": np.ascontiguousarray(wo_c),
        })
    return in_maps


def kernel(q, k, v, wq, bq, wk, bk, wv, bv, wo, bo):
    global last_results
    B = np.asarray(q).shape[0]
    nc = _get_nc()
    in_maps = prepare_in_maps(q, k, v, wq, bq, wk, bk, wv, bv, wo, bo)

    trace = bool(int(os.environ.get("KERNEL_TRACE", "0")))
    res = run_bass_kernel_spmd(nc, in_maps, list(range(N_CORES)), trace=trace)
    last_results = res

    attn = np.empty((B, NH, S, S), np.float32)
    out = np.zeros((B, S, DM), np.float32)
    for c in range(N_CORES):
        b, g = c // 4, c % 4
        r = res.results[c]
        at = r["attnT"]
        for h in range(HPC):
            attn[b, g * HPC + h] = at[h].T
        out[b] += r["outp"]
    return out, attn


# revision 9
# speedup vs baseline: 2.0869x; 2.0869x over previous
"""Multi-head attention (B=2, S=2048, D=1024, H=16) on 8 trn2 NeuronCores.

Sharding: (batch, head-group) -> core.  Core c handles batch b=c//4 and the
4 heads [4*(c%4), 4*(c%4)+4).  Per core:
  - QKV projections for its head slice (bias folded in via an augmented
    ones-row matmul; the 1/sqrt(depth) logit scale folded into wq).
  - Attention with logits computed TRANSPOSED (logitsT[k,q]) so that
    PT = exp(logitsT) feeds the ctx matmul directly as the stationary
    operand with zero on-chip transposes.
  - Softmax denominators come for free from a ones-column appended to V
    (row 64 of the ctx accumulator = sum_k exp(logits[k,q])).
  - attn probabilities are written as attnT[h,k,q] (perfectly contiguous
    DMA); the host transposes at gather time.
  - Output projection produces a partial out (this core's heads); host sums
    the 4 partials per batch.
"""

import os
import sys
from contextlib import ExitStack

import numpy as np

ONE_F32_BITS = 0x3F800000

for _p in ("/opt/trn_rl_repo", "/root/.axon_site/_ro/trn_rl_repo"):
    if os.path.isdir(_p) and _p not in sys.path:
        sys.path.insert(0, _p)

import concourse.bass as bass  # noqa: E402
import concourse.mybir as mybir  # noqa: E402
import concourse.tile as tile  # noqa: E402
from concourse import bacc  # noqa: E402
from concourse.bass_utils import run_bass_kernel_spmd  # noqa: E402

F32 = mybir.dt.float32
F32R = mybir.dt.float32r


def _r(ap):
    return ap.bitcast(F32R)

EXP = mybir.ActivationFunctionType.Exp
COPY = mybir.ActivationFunctionType.Copy

S = 2048          # sequence length
DM = 1024         # d_model
DH = 64           # depth per head
NH = 16           # total heads
HPC = 4           # heads per core
KT = DM // 128    # 8 k-tiles over d_model
NS = S // 128     # 16 s-tiles (128-row blocks)
QH = S // 2       # q processed in two halves of 1024
N_CORES = 8

_compiled_nc = None
last_results = None  # BassKernelResults of the most recent run (for test.py)


def _build(reps=1):
    nc = bacc.Bacc(None)

    xqT = nc.dram_tensor("xqT", [DM, S], F32R, kind="ExternalInput")
    xkT = nc.dram_tensor("xkT", [DM, S], F32R, kind="ExternalInput")
    xvT = nc.dram_tensor("xvT", [DM, S], F32R, kind="ExternalInput")
    wq_s = nc.dram_tensor("wq_s", [DM + 1, HPC * DH], F32R, kind="ExternalInput")
    wk_s = nc.dram_tensor("wk_s", [DM + 1, HPC * DH], F32R, kind="ExternalInput")
    wv_s = nc.dram_tensor("wv_s", [DM + 1, HPC * DH], F32R, kind="ExternalInput")
    wo_s = nc.dram_tensor("wo_s", [HPC * DH + 1, DM], F32R, kind="ExternalInput")
    attnT = nc.dram_tensor("attnT", [HPC, S, S], F32, kind="ExternalOutput")
    outp = nc.dram_tensor("outp", [S, DM], F32, kind="ExternalOutput")
    qk_spill = nc.dram_tensor("qk_spill", [2, HPC, DH, S], F32R)

    with tile.TileContext(nc) as tc:
      for _rep in range(reps):
       with ExitStack() as ctx:
        # ---------- persistent pools ----------
        vones_p = ctx.enter_context(tc.tile_pool(name="vones", bufs=NS))
        ctxT_p = ctx.enter_context(tc.tile_pool(name="ctxT", bufs=2))
        ones_p = ctx.enter_context(tc.tile_pool(name="ones", bufs=1))

        # V (natural layout) with a ones column per head: [128, 4*(64+1)]
        vones = [vones_p.tile([128, HPC * (DH + 1)], F32R, tag="vones", name="vones")
                 for _ in range(NS)]
        # ctxT (unnormalized attn-weighted values, transposed): 2 x [128, S]
        ctxT = [ctxT_p.tile([128, S], F32R, tag="ctxT", name="ctxT") for _ in range(2)]
        ones1 = ones_p.tile([1, 128], F32R, tag="ones", name="ones1")
        nc.vector.memset(ones1.bitcast(mybir.dt.uint32), ONE_F32_BITS)
        ones_f = ones_p.tile([1, 128], F32, tag="onesf", name="ones_f")
        nc.vector.memset(ones_f, 1.0)

        # ---------- phase 1: projections ----------
        with tc.tile_pool(name="xT", bufs=9) as xT_p, \
             tc.tile_pool(name="wt", bufs=10) as wt_p, \
             tc.tile_pool(name="stg", bufs=4) as stg_p, \
             tc.tile_pool(name="orow", bufs=1) as orow_p, \
             tc.tile_pool(name="pj", bufs=8, space="PSUM") as pj_ps:

            ones_row = orow_p.tile([1, S], F32R, tag="orow", name="ones_row")
            nc.vector.memset(ones_row.bitcast(mybir.dt.uint32), ONE_F32_BITS)

            # --- Q and K projections -> qhT/khT [64, S] per head, spilled ---
            for which, xsrc, wsrc in ((0, xqT, wq_s), (1, xkT, wk_s)):
                wts, xts = [], []
                for kk in range(KT + 1):
                    rows = 128 if kk < KT else 1
                    wt = wt_p.tile([rows, HPC * DH], F32R, tag="wt", name="wt")
                    nc.sync.dma_start(out=wt, in_=wsrc[kk * 128:kk * 128 + rows, :])
                    wts.append(wt)
                for kk in range(KT):
                    xt = xT_p.tile([128, S], F32R, tag="xT", name="xt")
                    nc.sync.dma_start(out=xt, in_=xsrc[kk * 128:(kk + 1) * 128, :])
                    xts.append(xt)
                xts.append(ones_row)

                pss = [[pj_ps.tile([128, 512], F32, tag="pj", name="pjq") for _ in range(4)]
                       for _ in range(2)]
                for kk in range(KT + 1):
                    for m in range(2):
                        for n in range(4):
                            nc.tensor.matmul(
                                pss[m][n],
                                lhsT=wts[kk][:, m * 128:(m + 1) * 128],
                                rhs=xts[kk][:, n * 512:(n + 1) * 512],
                                start=(kk == 0), stop=(kk == KT))
                stgs = [stg_p.tile([DH, S], F32R, tag="stg", name="stg") for _ in range(HPC)]
                for m in range(2):
                    for n in range(4):
                        ns_ = slice(n * 512, (n + 1) * 512)
                        nc.vector.tensor_copy(stgs[2 * m][:, ns_], pss[m][n][0:DH, :])
                        nc.vector.tensor_copy(stgs[2 * m + 1][:, ns_], pss[m][n][DH:128, :])
                for h in range(HPC):
                    nc.gpsimd.dma_start(out=qk_spill[which, h], in_=stgs[h])

            # --- V projection -> vones tiles (natural [S, 4*64] + ones) ---
            wvs, xvs = [], []
            for kk in range(KT + 1):
                rows = 128 if kk < KT else 1
                wt = wt_p.tile([rows, HPC * DH], F32R, tag="wt", name="wt")
                nc.sync.dma_start(out=wt, in_=wv_s[kk * 128:kk * 128 + rows, :])
                wvs.append(wt)
            for kk in range(KT):
                xt = xT_p.tile([128, S], F32R, tag="xT", name="xt")
                nc.sync.dma_start(out=xt, in_=xvT[kk * 128:(kk + 1) * 128, :])
                xvs.append(xt)
            xvs.append(ones_row)

            for s in range(NS):
                nc.vector.memset(vones[s].bitcast(mybir.dt.uint32), ONE_F32_BITS)
                ps = pj_ps.tile([128, HPC * DH], F32, tag="pj", name="pjv")
                for kk in range(KT + 1):
                    nc.tensor.matmul(
                        ps,
                        lhsT=xvs[kk][:, s * 128:(s + 1) * 128],
                        rhs=wvs[kk],
                        start=(kk == 0), stop=(kk == KT))
                dst = vones[s].rearrange("p (h c) -> p h c", c=DH + 1)[:, :, 0:DH]
                src = ps.rearrange("p (h c) -> p h c", c=DH)
                nc.vector.tensor_copy(dst, src)

        # ---------- phase 2: attention ----------
        with tc.tile_pool(name="qk", bufs=4) as qk_p, \
             tc.tile_pool(name="PT", bufs=16) as PT_p, \
             tc.tile_pool(name="rows", bufs=2) as rows_p, \
             tc.tile_pool(name="bc", bufs=2) as bc_p, \
             tc.tile_pool(name="lg", bufs=2, space="PSUM") as lg_ps, \
             tc.tile_pool(name="cx", bufs=2, space="PSUM") as cx_ps:

            for h in range(HPC):
                qh = qk_p.tile([DH, S], F32R, tag="qk", name="qk")
                kh = qk_p.tile([DH, S], F32R, tag="qk", name="qk")
                nc.sync.dma_start(out=qh, in_=qk_spill[0, h])
                nc.sync.dma_start(out=kh, in_=qk_spill[1, h])
                for q2 in range(2):
                    q0 = q2 * QH
                    cx = cx_ps.tile([DH + 1, QH], F32, tag="cx", name="cx")
                    pts = []
                    for kb in range(NS):
                        lg = lg_ps.tile([128, QH], F32, tag="lg", name="lg")
                        for n in range(2):
                            nc.tensor.matmul(
                                lg[:, n * 512:(n + 1) * 512],
                                lhsT=kh[:, kb * 128:(kb + 1) * 128],
                                rhs=qh[:, q0 + n * 512:q0 + (n + 1) * 512],
                                start=True, stop=True)
                        pt = PT_p.tile([128, QH], F32R, tag="PT", name="pt")
                        nc.scalar.activation(pt, lg, EXP)
                        for n in range(2):
                            nc.tensor.matmul(
                                cx[:, n * 512:(n + 1) * 512],
                                lhsT=vones[kb][:, h * (DH + 1):(h + 1) * (DH + 1)],
                                rhs=pt[:, n * 512:(n + 1) * 512],
                                start=(kb == 0), stop=(kb == NS - 1))
                        pts.append(pt)

                    # denominators -> reciprocal row -> broadcast tile
                    rcp = rows_p.tile([1, QH], F32, tag="rows", name="rcp")
                    nc.vector.reciprocal(rcp, cx[DH:DH + 1, :])
                    bps = lg_ps.tile([128, QH], F32, tag="lg", name="lg")
                    for n in range(2):
                        nc.tensor.matmul(
                            bps[:, n * 512:(n + 1) * 512],
                            lhsT=ones_f,
                            rhs=rcp[:, n * 512:(n + 1) * 512],
                            start=True, stop=True)
                    bc = bc_p.tile([128, QH], F32, tag="bc", name="bc")
                    nc.vector.tensor_copy(bc, bps)

                    # ctxT rows: fused normalize + copyback into resident tile
                    t, r0 = h // 2, (h % 2) * DH
                    nc.vector.tensor_mul(
                        ctxT[t][r0:r0 + DH, q0:q0 + QH], cx[0:DH, :], bc[0:DH, :])

                    # normalize PT in place and stream out
                    for kb in range(NS):
                        nc.vector.tensor_mul(pts[kb], pts[kb], bc)
                        nc.gpsimd.dma_start(
                            out=attnT[h, kb * 128:(kb + 1) * 128, q0:q0 + QH],
                            in_=pts[kb])

        # ---------- phase 3: output projection (partial over this core's heads) ----------
        with tc.tile_pool(name="wo", bufs=3) as wo_p, \
             tc.tile_pool(name="ost", bufs=3) as ost_p, \
             tc.tile_pool(name="ops", bufs=4, space="PSUM") as o_ps:
            wo0 = wo_p.tile([128, DM], F32R, tag="wo", name="wo")
            wo1 = wo_p.tile([128, DM], F32R, tag="wo", name="wo")
            wob = wo_p.tile([1, DM], F32R, tag="wo", name="wob")
            nc.sync.dma_start(out=wo0, in_=wo_s[0:128, :])
            nc.sync.dma_start(out=wo1, in_=wo_s[128:256, :])
            nc.sync.dma_start(out=wob, in_=wo_s[256:257, :])

            for m in range(NS):
                ms = slice(m * 128, (m + 1) * 128)
                ost = ost_p.tile([128, DM], F32, tag="ost", name="ost")
                pss = [o_ps.tile([128, 512], F32, tag="ops", name="ops") for _ in range(2)]
                for ti, (lhs_full, rhs_t) in enumerate(
                        ((ctxT[0], wo0), (ctxT[1], wo1), (ones1, wob))):
                    lhs = lhs_full if ti == 2 else lhs_full[:, ms]
                    for n in range(2):
                        nc.tensor.matmul(
                            pss[n],
                            lhsT=lhs,
                            rhs=rhs_t[:, n * 512:(n + 1) * 512],
                            start=(ti == 0), stop=(ti == 2))
                for n in range(2):
                    nc.scalar.activation(ost[:, n * 512:(n + 1) * 512], pss[n], COPY)
                nc.gpsimd.dma_start(out=outp[ms, :], in_=ost)

    nc.compile()
    return nc


def _get_nc():
    global _compiled_nc
    if _compiled_nc is None:
        _compiled_nc = _build()
    return _compiled_nc


def prepare_in_maps(q, k, v, wq, bq, wk, bk, wv, bv, wo, bo):
    q = np.asarray(q, np.float32)
    k = np.asarray(k, np.float32)
    v = np.asarray(v, np.float32)
    wq = np.asarray(wq, np.float32)
    bq = np.asarray(bq, np.float32)
    wk = np.asarray(wk, np.float32)
    bk = np.asarray(bk, np.float32)
    wv = np.asarray(wv, np.float32)
    bv = np.asarray(bv, np.float32)
    wo = np.asarray(wo, np.float32)
    bo = np.asarray(bo, np.float32)
    B = q.shape[0]
    scale = np.float32(1.0 / np.sqrt(DH))

    xT = {}
    for b in range(B):
        xT[("q", b)] = np.ascontiguousarray(q[b].T)
        xT[("k", b)] = np.ascontiguousarray(k[b].T)
        xT[("v", b)] = np.ascontiguousarray(v[b].T)

    in_maps = []
    for c in range(N_CORES):
        b, g = c // 4, c % 4
        col = g * HPC * DH
        cs = slice(col, col + HPC * DH)
        wq_c = np.concatenate([wq[:, cs], bq[None, cs]], 0) * scale
        wk_c = np.concatenate([wk[:, cs], bk[None, cs]], 0)
        wv_c = np.concatenate([wv[:, cs], bv[None, cs]], 0)
        bo_row = bo[None, :] if g == 0 else np.zeros((1, DM), np.float32)
        wo_c = np.concatenate([wo[cs, :], bo_row], 0)
        in_maps.append({
            "xqT": xT[("q", b)],
            "xkT": xT[("k", b)],
            "xvT": xT[("v", b)],
            "wq_s": np.ascontiguousarray(wq_c),
            "wk_s": np.ascontiguousarray(wk_c),
            "wv_s": np.ascontiguousarray(wv_c),
            "wo_s": np.ascontiguousarray(wo_c),
        })
    return in_maps


def kernel(q, k, v, wq, bq, wk, bk, wv, bv, wo, bo):
    global last_results
    B = np.asarray(q).shape[0]
    nc = _get_nc()
    in_maps = prepare_in_maps(q, k, v, wq, bq, wk, bk, wv, bv, wo, bo)

    trace = bool(int(os.environ.get("KERNEL_TRACE", "0")))
    res = run_bass_kernel_spmd(nc, in_maps, list(range(N_CORES)), trace=trace)
    last_results = res

    attn = np.empty((B, NH, S, S), np.float32)
    out = np.zeros((B, S, DM), np.float32)
    for c in range(N_CORES):
        b, g = c // 4, c % 4
        r = res.results[c]
        at = r["attnT"]
        for h in range(HPC):
            attn[b, g * HPC + h] = at[h].T
        out[b] += r["outp"]
    return out, attn
